# revision 4
# baseline (speedup 1.0000x reference)
"""Multi-head self-attention TRN2 kernel.

Sharding (8 cores): core c = (b, hg) with b = c // 4 (batch), hg = c % 4
(head group of 4 heads = 512 feature slice). Each core:
  - computes K^T, V projections for its 4 heads over its batch (phase A)
  - computes Q^T + flash-style attention per head, normalized O^T (phase B)
  - per-head AllGather of O^T across the 4 cores of its batch group
  - out-projection for its 512-column output slice + bo (phase C)
Host assembles the two batches x four column slices (pure concatenation).

All matmuls run in float32r (full PE rate, ~2e-4 rel err). Softmax skips the
max-subtraction: scores*scale for this problem are O(1), far from exp range
limits, and softmax is shift-invariant.
"""

import sys

sys.path.insert(0, "/opt/trn_rl_repo")

import numpy as np

import concourse.bass as bass
import concourse.mybir as mybir
import concourse.tile as tile
from concourse.bass_utils import run_bass_kernel_spmd

F32 = mybir.dt.float32
F32R = mybir.dt.float32r
ID = mybir.ActivationFunctionType.Identity
EXP = mybir.ActivationFunctionType.Exp

P = 128          # partitions
D = 2048         # hidden
S = 2048         # sequence
B = 2            # batch
HEADS_PER_CORE = 4
E = 512          # feature slice per core (4 heads * 128)
DH = 128         # head dim
ST = 512         # s-tile width
N_ST = S // ST           # 4 s-tiles
N_DC = D // P            # 16 contraction chunks
N_TC = S // P            # 16 t-chunks (keys)
N_SS = S // P            # 16 s-strips (phase C)
SCALE = 1.0 / np.sqrt(DH)

_CACHE = {}


def _install_ntff_hook():
    """Recreate the missing antenv.axon_hooks module so trace=True works.

    The agent image's antenv lacks axon_hooks; trn_boot degrades silently.
    This registers the same ctypes-based NTFF profile hook against
    /opt/axon/libaxon_pjrt.so.
    """
    import types
    import ctypes
    import contextlib

    if "antenv.axon_hooks" in sys.modules:
        return
    so_path = "/opt/axon/libaxon_pjrt.so"
    lib = ctypes.CDLL(so_path)
    if not hasattr(lib, "axon_start_nrt_profile"):
        return
    lib.axon_start_nrt_profile.argtypes = [
        ctypes.POINTER(ctypes.c_int64), ctypes.c_size_t]
    lib.axon_start_nrt_profile.restype = ctypes.c_int64
    lib.axon_stop_nrt_profile.argtypes = [ctypes.c_char_p]
    lib.axon_stop_nrt_profile.restype = ctypes.c_int64

    @contextlib.contextmanager
    def _hook(output_dir, device_ids):
        import jax
        jax.devices()
        if device_ids:
            ids = (ctypes.c_int64 * len(device_ids))(*device_ids)
            rc = lib.axon_start_nrt_profile(ids, len(device_ids))
        else:
            rc = lib.axon_start_nrt_profile(None, 0)
        if rc != 0:
            raise RuntimeError(f"axon_start_nrt_profile rc={rc}")
        try:
            yield
        finally:
            n = lib.axon_stop_nrt_profile(str(output_dir).encode())
            print(f"profile: {n} file(s) written to {output_dir}",
                  file=sys.stderr)

    mod = types.ModuleType("antenv.axon_hooks")
    _state = {"hook": _hook}
    mod.set_axon_ntff_profile_hook = lambda h: _state.__setitem__("hook", h)
    mod.get_axon_ntff_profile_hook = lambda: _state["hook"]
    sys.modules["antenv.axon_hooks"] = mod
    import antenv
    antenv.axon_hooks = mod


def split_multi_waits(nc, limit=1):
    """This container's walrus accepts only `limit` sync waits per
    instruction; hoist extras onto single-wait NoOps on the same engine."""
    for fn in nc.m.functions:
        for bb in fn.blocks:
            new_insts = []
            for inst in bb.instructions:
                si = inst.sync_info
                nw = len(si.on_wait) if si and si.on_wait else 0
                if nw > limit:
                    waits = list(si.on_wait)
                    head, tail = waits[:-limit], waits[-limit:]
                    for j, w in enumerate(head):
                        nop = mybir.InstNoOp(
                            name=f"{inst.name}-wsplit{j}", ins=[], outs=[])
                        nop.engine = inst.engine
                        nop.sync_info = mybir.SyncInfo(on_wait=[w], on_update=[])
                        new_insts.append(nop)
                    inst.sync_info = mybir.SyncInfo(
                        on_wait=tail, on_update=list(si.on_update or []))
                new_insts.append(inst)
            bb.instructions = new_insts


def build_nc():
    nc = bass.Bass()

    xt_ext = nc.declare_dram_parameter("xt", [D, S], F32R, isOutput=False)
    wq_ext = nc.declare_dram_parameter("wq", [D, E], F32R, isOutput=False)
    wk_ext = nc.declare_dram_parameter("wk", [D, E], F32R, isOutput=False)
    wv_ext = nc.declare_dram_parameter("wv", [D, E], F32R, isOutput=False)
    wo_ext = nc.declare_dram_parameter("wo", [D, E], F32R, isOutput=False)
    bq_ext = nc.declare_dram_parameter("bq", [P, HEADS_PER_CORE], F32, isOutput=False)
    bk_ext = nc.declare_dram_parameter("bk", [P, HEADS_PER_CORE], F32, isOutput=False)
    bv_ext = nc.declare_dram_parameter("bv", [P, E], F32, isOutput=False)
    bo_ext = nc.declare_dram_parameter("bo", [P, E], F32, isOutput=False)
    onesc_ext = nc.declare_dram_parameter("onesc", [P, 1], F32R, isOutput=False)
    onesr_ext = nc.declare_dram_parameter("onesr", [1, P], F32R, isOutput=False)
    out_ext = nc.declare_dram_parameter("out", [S, E], F32, isOutput=True)

    xt_r = xt_ext.rearrange("(dc p) s -> p dc s", p=P)
    w_r = {
        "wq": wq_ext.rearrange("(dc p) e -> p dc e", p=P),
        "wk": wk_ext.rearrange("(dc p) e -> p dc e", p=P),
        "wv": wv_ext.rearrange("(dc p) e -> p dc e", p=P),
        "wo": wo_ext.rearrange("(dc p) e -> p dc e", p=P),
    }

    with tile.TileContext(nc) as tc:
        with tc.tile_pool(name="persist", bufs=1) as persist, \
             tc.tile_pool(name="xp", bufs=3) as xp, \
             tc.tile_pool(name="dram", bufs=1, space="DRAM") as dram:

            # ---- constants / biases ----
            bq_sb = persist.tile([P, HEADS_PER_CORE], F32)
            bk_sb = persist.tile([P, HEADS_PER_CORE], F32)
            bv_sb = persist.tile([P, E], F32)
            bo_sb = persist.tile([P, E], F32)
            onesc = persist.tile([P, 1], F32R)
            onesr = persist.tile([1, P], F32R)
            nc.sync.dma_start(bq_sb[:], bq_ext[:])
            nc.sync.dma_start(bk_sb[:], bk_ext[:])
            nc.sync.dma_start(bv_sb[:], bv_ext[:])
            nc.sync.dma_start(bo_sb[:], bo_ext[:])
            nc.sync.dma_start(onesc[:], onesc_ext[:])
            nc.sync.dma_start(onesr[:], onesr_ext[:])

            # ---- persistent activations ----
            k_sb = persist.tile([P, HEADS_PER_CORE, S], F32R)   # K^T [dh, h, t]
            v_sb = persist.tile([P, N_TC, E], F32R)             # V   [t-strip, tc, e]

            # AllGather buffers (one per head position)
            ag_in = [dram.tile([P, S], F32R, name=f"ag_in{h}")
                     for h in range(HEADS_PER_CORE)]
            ag_out = [dram.tile([4 * P, S], F32R, name=f"ag_out{h}")
                      for h in range(HEADS_PER_CORE)]

            # ================= Phase A: K^T and V projections =================
            with tc.tile_pool(name="wkv", bufs=1) as wkv, \
                 tc.tile_pool(name="psA", bufs=4, space="PSUM") as psA:
                wk_sb = wkv.tile([P, N_DC, E], F32R)
                wv_sb = wkv.tile([P, N_DC, E], F32R)
                nc.sync.dma_start(wk_sb[:], w_r["wk"])
                nc.sync.dma_start(wv_sb[:], w_r["wv"])

                for st in range(N_ST):
                    xt_lo = xp.tile([P, N_DC // 2, ST], F32R, tag="xt")
                    xt_hi = xp.tile([P, N_DC // 2, ST], F32R, tag="xt")
                    nc.sync.dma_start(xt_lo[:], xt_r[:, :N_DC // 2, st * ST:(st + 1) * ST])
                    nc.sync.dma_start(xt_hi[:], xt_r[:, N_DC // 2:, st * ST:(st + 1) * ST])

                    def xt_sb(dc, lo=xt_lo, hi=xt_hi):
                        return lo[:, dc] if dc < N_DC // 2 else hi[:, dc - N_DC // 2]

                    # K^T strips: out [e-strip 128, s 512]
                    for es in range(HEADS_PER_CORE):
                        psk = psA.tile([P, ST], F32, tag="psA")
                        for dc in range(N_DC):
                            nc.tensor.matmul(
                                psk[:], wk_sb[:, dc, es * P:(es + 1) * P],
                                xt_sb(dc),
                                start=(dc == 0), stop=(dc == N_DC - 1))
                        nc.scalar.activation(
                            k_sb[:, es, st * ST:(st + 1) * ST], psk[:],
                            ID, bias=bk_sb[:, es:es + 1], scale=1.0)
                    # V strips: out [t-strip 128, e 512]
                    for tl in range(ST // P):
                        ts_g = st * (ST // P) + tl
                        psv = psA.tile([P, E], F32, tag="psA")
                        for dc in range(N_DC):
                            nc.tensor.matmul(
                                psv[:], xt_sb(dc)[:, tl * P:(tl + 1) * P],
                                wv_sb[:, dc, :],
                                start=(dc == 0), stop=(dc == N_DC - 1))
                        with nc.allow_low_precision(reason="f32r V"):
                            nc.vector.tensor_add(
                                out=v_sb[:, ts_g, :], in0=psv[:], in1=bv_sb[:])

            # ============ Phase B: Q^T projection + attention per head ============
            with tc.tile_pool(name="wq", bufs=1) as wqp, \
                 tc.tile_pool(name="workB", bufs=2) as work, \
                 tc.tile_pool(name="psq", bufs=2, space="PSUM") as psq_pool, \
                 tc.tile_pool(name="psB", bufs=2, space="PSUM") as psB:
                wq_sb = wqp.tile([P, N_DC, E], F32R)
                nc.sync.dma_start(wq_sb[:], w_r["wq"])

                for st in range(N_ST):
                    xt_lo = xp.tile([P, N_DC // 2, ST], F32R, tag="xt")
                    xt_hi = xp.tile([P, N_DC // 2, ST], F32R, tag="xt")
                    nc.sync.dma_start(xt_lo[:], xt_r[:, :N_DC // 2, st * ST:(st + 1) * ST])
                    nc.sync.dma_start(xt_hi[:], xt_r[:, N_DC // 2:, st * ST:(st + 1) * ST])

                    def xt_sb(dc, lo=xt_lo, hi=xt_hi):
                        return lo[:, dc] if dc < N_DC // 2 else hi[:, dc - N_DC // 2]

                    q_st = work.tile([P, HEADS_PER_CORE, ST], F32R, tag="q")
                    for es in range(HEADS_PER_CORE):
                        psq = psq_pool.tile([P, ST], F32, tag="psq")
                        for dc in range(N_DC):
                            nc.tensor.matmul(
                                psq[:], wq_sb[:, dc, es * P:(es + 1) * P],
                                xt_sb(dc),
                                start=(dc == 0), stop=(dc == N_DC - 1))
                        nc.scalar.activation(
                            q_st[:, es, :], psq[:],
                            ID, bias=bq_sb[:, es:es + 1], scale=1.0)

                    for h in range(HEADS_PER_CORE):
                        pso = psB.tile([P, ST], F32, tag="o")
                        psd = psB.tile([1, ST], F32, tag="d")
                        for tcI in range(N_TC):
                            pss = psq_pool.tile([P, ST], F32, tag="psq")
                            nc.tensor.matmul(
                                pss[:], k_sb[:, h, tcI * P:(tcI + 1) * P],
                                q_st[:, h, :], start=True, stop=True)
                            pt = work.tile([P, ST], F32R, tag="pt")
                            nc.scalar.activation(pt[:], pss[:], EXP,
                                                 bias=0.0, scale=float(SCALE))
                            nc.tensor.matmul(
                                pso[:], v_sb[:, tcI, h * P:(h + 1) * P], pt[:],
                                start=(tcI == 0), stop=(tcI == N_TC - 1))
                            nc.tensor.matmul(
                                psd[:], onesc[:], pt[:],
                                start=(tcI == 0), stop=(tcI == N_TC - 1))
                        recip = work.tile([1, ST], F32R, tag="recip")
                        with nc.allow_low_precision(reason="softmax recip"):
                            nc.vector.reciprocal(recip[:], psd[:])
                        rb_ps = psB.tile([P, ST], F32, tag="rb")
                        nc.tensor.matmul(rb_ps[:], onesr[:], recip[:],
                                         start=True, stop=True)
                        rb_sb = work.tile([P, ST], F32, tag="rb_sb")
                        nc.vector.tensor_copy(rb_sb[:], rb_ps[:])
                        o_sb = work.tile([P, ST], F32R, tag="o_sb")
                        with nc.allow_low_precision(reason="f32r O"):
                            nc.vector.tensor_mul(out=o_sb[:], in0=pso[:], in1=rb_sb[:])
                        nc.sync.dma_start(
                            ag_in[h][:, st * ST:(st + 1) * ST], o_sb[:])

                # per-head AllGather across the batch group
                for h in range(HEADS_PER_CORE):
                    nc.gpsimd.collective_compute(
                        "AllGather", mybir.AluOpType.bypass,
                        ins=[ag_in[h][:]], outs=[ag_out[h][:]],
                        replica_groups=[[0, 1, 2, 3], [4, 5, 6, 7]],
                    )

            # ================= Phase C: out projection =================
            with tc.tile_pool(name="wo", bufs=1) as wop, \
                 tc.tile_pool(name="workC", bufs=2) as work, \
                 tc.tile_pool(name="psC", bufs=4, space="PSUM") as psC:
                wo_sb = wop.tile([P, N_DC, E], F32R)
                nc.sync.dma_start(wo_sb[:], w_r["wo"])
                ag_r = [ag_out[h].rearrange("(g p) s -> p g s", p=P)
                        for h in range(HEADS_PER_CORE)]
                for ss in range(N_SS):
                    of_sb = work.tile([P, HEADS_PER_CORE, 4, P], F32R, tag="of")
                    for h in range(HEADS_PER_CORE):
                        nc.sync.dma_start(
                            of_sb[:, h, :, :],
                            ag_r[h][:, :, ss * P:(ss + 1) * P])
                    psc = psC.tile([P, E], F32, tag="psC")
                    for ec in range(N_DC):
                        hg, hh = ec // 4, ec % 4
                        nc.tensor.matmul(
                            psc[:], of_sb[:, hh, hg, :], wo_sb[:, ec, :],
                            start=(ec == 0), stop=(ec == N_DC - 1))
                    out_sb = work.tile([P, E], F32, tag="out_sb")
                    nc.vector.tensor_add(out=out_sb[:], in0=psc[:], in1=bo_sb[:])
                    nc.sync.dma_start(out_ext[ss * P:(ss + 1) * P, :], out_sb[:])

    split_multi_waits(nc)
    return nc


def _get_nc():
    if "nc" not in _CACHE:
        _CACHE["nc"] = build_nc()
    return _CACHE["nc"]


def _prep_in_maps(X, Wq, bq, Wk, bk, Wv, bv, Wo, bo):
    xt = [np.ascontiguousarray(X[b].T) for b in range(B)]  # [d, s]
    onesc = np.ones((P, 1), np.float32)
    onesr = np.ones((1, P), np.float32)
    in_maps = []
    for c in range(8):
        b, hg = c // 4, c % 4
        sl = slice(hg * E, (hg + 1) * E)
        in_maps.append({
            "xt": xt[b],
            "wq": np.ascontiguousarray(Wq[sl, :].T),
            "wk": np.ascontiguousarray(Wk[sl, :].T),
            "wv": np.ascontiguousarray(Wv[sl, :].T),
            "wo": np.ascontiguousarray(Wo[sl, :].T),
            "bq": np.ascontiguousarray(bq[sl].reshape(HEADS_PER_CORE, P).T),
            "bk": np.ascontiguousarray(bk[sl].reshape(HEADS_PER_CORE, P).T),
            "bv": np.broadcast_to(bv[sl], (P, E)).copy(),
            "bo": np.broadcast_to(bo[sl], (P, E)).copy(),
            "onesc": onesc,
            "onesr": onesr,
        })
    return in_maps


def kernel(X, Wq, bq, Wk, bk, Wv, bv, Wo, bo, _trace=False):
    X = np.asarray(X, dtype=np.float32)
    Wq = np.asarray(Wq, dtype=np.float32)
    bq = np.asarray(bq, dtype=np.float32)
    Wk = np.asarray(Wk, dtype=np.float32)
    bk = np.asarray(bk, dtype=np.float32)
    Wv = np.asarray(Wv, dtype=np.float32)
    bv = np.asarray(bv, dtype=np.float32)
    Wo = np.asarray(Wo, dtype=np.float32)
    bo = np.asarray(bo, dtype=np.float32)

    nc = _get_nc()
    in_maps = _prep_in_maps(X, Wq, bq, Wk, bk, Wv, bv, Wo, bo)
    if _trace:
        _install_ntff_hook()
    res = run_bass_kernel_spmd(nc, in_maps, core_ids=list(range(8)),
                               trace=_trace)
    if _trace:
        _CACHE["last_results"] = res

    out = np.empty((B, S, D), dtype=np.float32)
    for c in range(8):
        b, hg = c // 4, c % 4
        out[b, :, hg * E:(hg + 1) * E] = res.results[c]["out"]
    return out


# revision 7
# speedup vs baseline: 1.0300x; 1.0300x over previous
"""Multi-head self-attention TRN2 kernel.

Sharding (8 cores): core c = (b, hg) with b = c // 4 (batch), hg = c % 4
(head group of 4 heads = 512 feature slice). Each core:
  - computes K^T, V projections for its 4 heads over its batch (phase A)
  - computes Q^T + flash-style attention per head, normalized O^T (phase B)
  - per-head AllGather of O^T across the 4 cores of its batch group
  - out-projection for its 512-column output slice + bo (phase C)
Host assembles the two batches x four column slices (pure concatenation).

All matmuls run in float32r (full PE rate, ~2e-4 rel err). Softmax skips the
max-subtraction: scores*scale for this problem are O(1), far from exp range
limits, and softmax is shift-invariant.
"""

import sys

sys.path.insert(0, "/opt/trn_rl_repo")

import numpy as np

import concourse.bass as bass
import concourse.mybir as mybir
import concourse.tile as tile
from concourse.bass_utils import run_bass_kernel_spmd

F32 = mybir.dt.float32
F32R = mybir.dt.float32r
ID = mybir.ActivationFunctionType.Identity
EXP = mybir.ActivationFunctionType.Exp

P = 128          # partitions
D = 2048         # hidden
S = 2048         # sequence
B = 2            # batch
HEADS_PER_CORE = 4
E = 512          # feature slice per core (4 heads * 128)
DH = 128         # head dim
ST = 512         # s-tile width
N_ST = S // ST           # 4 s-tiles
N_DC = D // P            # 16 contraction chunks
N_TC = S // P            # 16 t-chunks (keys)
N_SS = S // P            # 16 s-strips (phase C)
SCALE = 1.0 / np.sqrt(DH)

_CACHE = {}


def _install_ntff_hook():
    """Recreate the missing antenv.axon_hooks module so trace=True works.

    The agent image's antenv lacks axon_hooks; trn_boot degrades silently.
    This registers the same ctypes-based NTFF profile hook against
    /opt/axon/libaxon_pjrt.so.
    """
    import types
    import ctypes
    import contextlib

    if "antenv.axon_hooks" in sys.modules:
        return
    so_path = "/opt/axon/libaxon_pjrt.so"
    lib = ctypes.CDLL(so_path)
    if not hasattr(lib, "axon_start_nrt_profile"):
        return
    lib.axon_start_nrt_profile.argtypes = [
        ctypes.POINTER(ctypes.c_int64), ctypes.c_size_t]
    lib.axon_start_nrt_profile.restype = ctypes.c_int64
    lib.axon_stop_nrt_profile.argtypes = [ctypes.c_char_p]
    lib.axon_stop_nrt_profile.restype = ctypes.c_int64

    @contextlib.contextmanager
    def _hook(output_dir, device_ids):
        import jax
        jax.devices()
        if device_ids:
            ids = (ctypes.c_int64 * len(device_ids))(*device_ids)
            rc = lib.axon_start_nrt_profile(ids, len(device_ids))
        else:
            rc = lib.axon_start_nrt_profile(None, 0)
        if rc != 0:
            raise RuntimeError(f"axon_start_nrt_profile rc={rc}")
        try:
            yield
        finally:
            n = lib.axon_stop_nrt_profile(str(output_dir).encode())
            print(f"profile: {n} file(s) written to {output_dir}",
                  file=sys.stderr)

    mod = types.ModuleType("antenv.axon_hooks")
    _state = {"hook": _hook}
    mod.set_axon_ntff_profile_hook = lambda h: _state.__setitem__("hook", h)
    mod.get_axon_ntff_profile_hook = lambda: _state["hook"]
    sys.modules["antenv.axon_hooks"] = mod
    import antenv
    antenv.axon_hooks = mod


def split_multi_waits(nc, limit=1):
    """This container's walrus accepts only `limit` sync waits per
    instruction; hoist extras onto single-wait NoOps on the same engine."""
    for fn in nc.m.functions:
        for bb in fn.blocks:
            new_insts = []
            for inst in bb.instructions:
                si = inst.sync_info
                nw = len(si.on_wait) if si and si.on_wait else 0
                if nw > limit:
                    waits = list(si.on_wait)
                    head, tail = waits[:-limit], waits[-limit:]
                    for j, w in enumerate(head):
                        nop = mybir.InstNoOp(
                            name=f"{inst.name}-wsplit{j}", ins=[], outs=[])
                        nop.engine = inst.engine
                        nop.sync_info = mybir.SyncInfo(on_wait=[w], on_update=[])
                        new_insts.append(nop)
                    inst.sync_info = mybir.SyncInfo(
                        on_wait=tail, on_update=list(si.on_update or []))
                new_insts.append(inst)
            bb.instructions = new_insts


def build_nc():
    nc = bass.Bass()

    xt_ext = nc.declare_dram_parameter("xt", [D, S], F32R, isOutput=False)
    wq_ext = nc.declare_dram_parameter("wq", [D, E], F32R, isOutput=False)
    wk_ext = nc.declare_dram_parameter("wk", [D, E], F32R, isOutput=False)
    wv_ext = nc.declare_dram_parameter("wv", [D, E], F32R, isOutput=False)
    wo_ext = nc.declare_dram_parameter("wo", [D, E], F32R, isOutput=False)
    bq_ext = nc.declare_dram_parameter("bq", [P, HEADS_PER_CORE], F32, isOutput=False)
    bk_ext = nc.declare_dram_parameter("bk", [P, HEADS_PER_CORE], F32, isOutput=False)
    bv_ext = nc.declare_dram_parameter("bv", [P, E], F32, isOutput=False)
    bo_ext = nc.declare_dram_parameter("bo", [P, E], F32, isOutput=False)
    onesc_ext = nc.declare_dram_parameter("onesc", [P, 1], F32R, isOutput=False)
    onesr_ext = nc.declare_dram_parameter("onesr", [1, P], F32R, isOutput=False)
    out_ext = nc.declare_dram_parameter("out", [S, E], F32, isOutput=True)

    xt_r = xt_ext.rearrange("(dc p) s -> p dc s", p=P)
    w_r = {
        "wq": wq_ext.rearrange("(dc p) e -> p dc e", p=P),
        "wk": wk_ext.rearrange("(dc p) e -> p dc e", p=P),
        "wv": wv_ext.rearrange("(dc p) e -> p dc e", p=P),
        "wo": wo_ext.rearrange("(dc p) e -> p dc e", p=P),
    }

    with tile.TileContext(nc) as tc:
        with tc.tile_pool(name="persist", bufs=1) as persist, \
             tc.tile_pool(name="xp", bufs=4) as xp, \
             tc.tile_pool(name="dram", bufs=1, space="DRAM") as dram:

            # ---- constants / biases ----
            bq_sb = persist.tile([P, HEADS_PER_CORE], F32)
            bk_sb = persist.tile([P, HEADS_PER_CORE], F32)
            bv_sb = persist.tile([P, E], F32)
            bo_sb = persist.tile([P, E], F32)
            onesc = persist.tile([P, 1], F32R)
            onesr = persist.tile([1, P], F32R)
            nc.sync.dma_start(bq_sb[:], bq_ext[:])
            nc.sync.dma_start(bk_sb[:], bk_ext[:])
            nc.sync.dma_start(bv_sb[:], bv_ext[:])
            nc.sync.dma_start(bo_sb[:], bo_ext[:])
            nc.sync.dma_start(onesc[:], onesc_ext[:])
            nc.sync.dma_start(onesr[:], onesr_ext[:])

            # ---- persistent activations ----
            k_sb = persist.tile([P, HEADS_PER_CORE, S], F32R)   # K^T [dh, h, t]
            v_sb = persist.tile([P, N_TC, E], F32R)             # V   [t-strip, tc, e]

            # AllGather buffers (one per head position)
            ag_in = [dram.tile([P, S], F32R, name=f"ag_in{h}")
                     for h in range(HEADS_PER_CORE)]
            ag_out = [dram.tile([4 * P, S], F32R, name=f"ag_out{h}")
                      for h in range(HEADS_PER_CORE)]

            # ================= Phase A: K^T and V projections =================
            with tc.tile_pool(name="wkv", bufs=1) as wkv, \
                 tc.tile_pool(name="psA", bufs=4, space="PSUM") as psA:
                wk_sb = wkv.tile([P, N_DC, E], F32R)
                wv_sb = wkv.tile([P, N_DC, E], F32R)
                nc.sync.dma_start(wk_sb[:], w_r["wk"])
                nc.sync.dma_start(wv_sb[:], w_r["wv"])

                for st in range(N_ST):
                    xt_lo = xp.tile([P, N_DC // 2, ST], F32R, tag="xt")
                    xt_hi = xp.tile([P, N_DC // 2, ST], F32R, tag="xt")
                    nc.sync.dma_start(xt_lo[:], xt_r[:, :N_DC // 2, st * ST:(st + 1) * ST])
                    nc.sync.dma_start(xt_hi[:], xt_r[:, N_DC // 2:, st * ST:(st + 1) * ST])

                    def xt_sb(dc, lo=xt_lo, hi=xt_hi):
                        return lo[:, dc] if dc < N_DC // 2 else hi[:, dc - N_DC // 2]

                    # K^T strips: out [e-strip 128, s 512]
                    for es in range(HEADS_PER_CORE):
                        psk = psA.tile([P, ST], F32, tag="psA")
                        for dc in range(N_DC):
                            nc.tensor.matmul(
                                psk[:], wk_sb[:, dc, es * P:(es + 1) * P],
                                xt_sb(dc),
                                start=(dc == 0), stop=(dc == N_DC - 1))
                        nc.scalar.activation(
                            k_sb[:, es, st * ST:(st + 1) * ST], psk[:],
                            ID, bias=bk_sb[:, es:es + 1], scale=1.0)
                    # V strips: out [t-strip 128, e 512]
                    for tl in range(ST // P):
                        ts_g = st * (ST // P) + tl
                        psv = psA.tile([P, E], F32, tag="psA")
                        for dc in range(N_DC):
                            nc.tensor.matmul(
                                psv[:], xt_sb(dc)[:, tl * P:(tl + 1) * P],
                                wv_sb[:, dc, :],
                                start=(dc == 0), stop=(dc == N_DC - 1))
                        with nc.allow_low_precision(reason="f32r V"):
                            nc.vector.tensor_add(
                                out=v_sb[:, ts_g, :], in0=psv[:], in1=bv_sb[:])

            # ============ Phase B1: Q^T projection, spilled to DRAM ============
            q_dram = dram.tile([E, S], F32R, name="q_dram")
            with tc.tile_pool(name="wq", bufs=1) as wqp, \
                 tc.tile_pool(name="workB1", bufs=3) as workB1, \
                 tc.tile_pool(name="psq", bufs=3, space="PSUM") as psq_pool:
                wq_sb = wqp.tile([P, N_DC, E], F32R)
                nc.sync.dma_start(wq_sb[:], w_r["wq"])

                for st in range(N_ST):
                    xt_lo = xp.tile([P, N_DC // 2, ST], F32R, tag="xt")
                    xt_hi = xp.tile([P, N_DC // 2, ST], F32R, tag="xt")
                    nc.sync.dma_start(xt_lo[:], xt_r[:, :N_DC // 2, st * ST:(st + 1) * ST])
                    nc.sync.dma_start(xt_hi[:], xt_r[:, N_DC // 2:, st * ST:(st + 1) * ST])

                    def xt_sb(dc, lo=xt_lo, hi=xt_hi):
                        return lo[:, dc] if dc < N_DC // 2 else hi[:, dc - N_DC // 2]

                    for es in range(HEADS_PER_CORE):
                        psq = psq_pool.tile([P, ST], F32, tag="psq")
                        for dc in range(N_DC):
                            nc.tensor.matmul(
                                psq[:], wq_sb[:, dc, es * P:(es + 1) * P],
                                xt_sb(dc),
                                start=(dc == 0), stop=(dc == N_DC - 1))
                        q_stage = workB1.tile([P, ST], F32R, tag="qs")
                        nc.scalar.activation(
                            q_stage[:], psq[:],
                            ID, bias=bq_sb[:, es:es + 1], scale=1.0)
                        nc.sync.dma_start(
                            q_dram[es * P:(es + 1) * P, st * ST:(st + 1) * ST],
                            q_stage[:])

            # ===== Phase B2: attention, head-outer, pipelined normalize =====
            with tc.tile_pool(name="workB2", bufs=3) as work, \
                 tc.tile_pool(name="pssc", bufs=2, space="PSUM") as pssc, \
                 tc.tile_pool(name="psB", bufs=1, space="PSUM") as psB:
                pending = [None]

                def flush_pending():
                    if pending[0] is not None:
                        pending[0]()
                        pending[0] = None

                for h in range(HEADS_PER_CORE):
                    for st in range(N_ST):
                        q_t = work.tile([P, ST], F32R, tag="qb")
                        nc.sync.dma_start(
                            q_t[:],
                            q_dram[h * P:(h + 1) * P, st * ST:(st + 1) * ST])
                        pso = psB.tile([P, ST], F32, tag="o", bufs=2)
                        psd = psB.tile([1, ST], F32, tag="d", bufs=1)
                        for tc2 in range(N_TC // 2):
                            t0, t1 = 2 * tc2, 2 * tc2 + 1
                            pss = pssc.tile([P, 2 * ST], F32, tag="sc")
                            nc.tensor.matmul(
                                pss[:, :ST], k_sb[:, h, t0 * P:(t0 + 1) * P],
                                q_t[:], start=True, stop=True)
                            nc.tensor.matmul(
                                pss[:, ST:], k_sb[:, h, t1 * P:(t1 + 1) * P],
                                q_t[:], start=True, stop=True)
                            pt = work.tile([P, 2 * ST], F32R, tag="pt")
                            nc.scalar.activation(pt[:], pss[:], EXP,
                                                 bias=0.0, scale=float(SCALE))
                            nc.tensor.matmul(
                                pso[:], v_sb[:, t0, h * P:(h + 1) * P],
                                pt[:, :ST],
                                start=(tc2 == 0), stop=False)
                            nc.tensor.matmul(
                                pso[:], v_sb[:, t1, h * P:(h + 1) * P],
                                pt[:, ST:],
                                start=False, stop=(tc2 == N_TC // 2 - 1))
                            nc.tensor.matmul(
                                psd[:], onesc[:], pt[:, :ST],
                                start=(tc2 == 0), stop=False)
                            nc.tensor.matmul(
                                psd[:], onesc[:], pt[:, ST:],
                                start=False, stop=(tc2 == N_TC // 2 - 1))
                        # reciprocal now (DVE, overlaps next block's PE work);
                        # defer broadcast+multiply one block
                        dsb = work.tile([1, ST], F32R, tag="dsb")
                        with nc.allow_low_precision(reason="softmax recip"):
                            nc.vector.reciprocal(dsb[:], psd[:])
                        flush_pending()

                        def normalize(h=h, st=st, pso=pso, dsb=dsb):
                            rb_ps = psB.tile([P, ST], F32, tag="rb", bufs=1)
                            nc.tensor.matmul(rb_ps[:], onesr[:], dsb[:],
                                             start=True, stop=True)
                            rb_sb = work.tile([P, ST], F32, tag="rb_sb")
                            nc.vector.tensor_copy(rb_sb[:], rb_ps[:])
                            o_sb = work.tile([P, ST], F32R, tag="o_sb")
                            with nc.allow_low_precision(reason="f32r O"):
                                nc.vector.tensor_mul(
                                    out=o_sb[:], in0=pso[:], in1=rb_sb[:])
                            nc.sync.dma_start(
                                ag_in[h][:, st * ST:(st + 1) * ST], o_sb[:])

                        pending[0] = normalize
                    # AllGather head h as soon as its last s-tile normalizes
                    flush_pending()
                    nc.gpsimd.collective_compute(
                        "AllGather", mybir.AluOpType.bypass,
                        ins=[ag_in[h][:]], outs=[ag_out[h][:]],
                        replica_groups=[[0, 1, 2, 3], [4, 5, 6, 7]],
                    )

            # ================= Phase C: out projection =================
            with tc.tile_pool(name="wo", bufs=1) as wop, \
                 tc.tile_pool(name="workC", bufs=2) as work, \
                 tc.tile_pool(name="psC", bufs=4, space="PSUM") as psC:
                wo_sb = wop.tile([P, N_DC, E], F32R)
                nc.sync.dma_start(wo_sb[:], w_r["wo"])
                ag_r = [ag_out[h].rearrange("(g p) s -> p g s", p=P)
                        for h in range(HEADS_PER_CORE)]
                for ss in range(N_SS):
                    of_sb = work.tile([P, HEADS_PER_CORE, 4, P], F32R, tag="of")
                    for h in range(HEADS_PER_CORE):
                        nc.sync.dma_start(
                            of_sb[:, h, :, :],
                            ag_r[h][:, :, ss * P:(ss + 1) * P])
                    psc = psC.tile([P, E], F32, tag="psC")
                    for ec in range(N_DC):
                        hg, hh = ec // 4, ec % 4
                        nc.tensor.matmul(
                            psc[:], of_sb[:, hh, hg, :], wo_sb[:, ec, :],
                            start=(ec == 0), stop=(ec == N_DC - 1))
                    out_sb = work.tile([P, E], F32, tag="out_sb")
                    nc.vector.tensor_add(out=out_sb[:], in0=psc[:], in1=bo_sb[:])
                    nc.sync.dma_start(out_ext[ss * P:(ss + 1) * P, :], out_sb[:])

    split_multi_waits(nc)
    return nc


def _get_nc():
    if "nc" not in _CACHE:
        _CACHE["nc"] = build_nc()
    return _CACHE["nc"]


def _prep_in_maps(X, Wq, bq, Wk, bk, Wv, bv, Wo, bo):
    xt = [np.ascontiguousarray(X[b].T) for b in range(B)]  # [d, s]
    onesc = np.ones((P, 1), np.float32)
    onesr = np.ones((1, P), np.float32)
    in_maps = []
    for c in range(8):
        b, hg = c // 4, c % 4
        sl = slice(hg * E, (hg + 1) * E)
        in_maps.append({
            "xt": xt[b],
            "wq": np.ascontiguousarray(Wq[sl, :].T),
            "wk": np.ascontiguousarray(Wk[sl, :].T),
            "wv": np.ascontiguousarray(Wv[sl, :].T),
            "wo": np.ascontiguousarray(Wo[sl, :].T),
            "bq": np.ascontiguousarray(bq[sl].reshape(HEADS_PER_CORE, P).T),
            "bk": np.ascontiguousarray(bk[sl].reshape(HEADS_PER_CORE, P).T),
            "bv": np.broadcast_to(bv[sl], (P, E)).copy(),
            "bo": np.broadcast_to(bo[sl], (P, E)).copy(),
            "onesc": onesc,
            "onesr": onesr,
        })
    return in_maps


def kernel(X, Wq, bq, Wk, bk, Wv, bv, Wo, bo, _trace=False):
    X = np.asarray(X, dtype=np.float32)
    Wq = np.asarray(Wq, dtype=np.float32)
    bq = np.asarray(bq, dtype=np.float32)
    Wk = np.asarray(Wk, dtype=np.float32)
    bk = np.asarray(bk, dtype=np.float32)
    Wv = np.asarray(Wv, dtype=np.float32)
    bv = np.asarray(bv, dtype=np.float32)
    Wo = np.asarray(Wo, dtype=np.float32)
    bo = np.asarray(bo, dtype=np.float32)

    nc = _get_nc()
    in_maps = _prep_in_maps(X, Wq, bq, Wk, bk, Wv, bv, Wo, bo)
    if _trace:
        _install_ntff_hook()
    res = run_bass_kernel_spmd(nc, in_maps, core_ids=list(range(8)),
                               trace=_trace)
    if _trace:
        _CACHE["last_results"] = res

    out = np.empty((B, S, D), dtype=np.float32)
    for c in range(8):
        b, hg = c // 4, c % 4
        out[b, :, hg * E:(hg + 1) * E] = res.results[c]["out"]
    return out


# revision 8
# speedup vs baseline: 1.3375x; 1.2985x over previous
"""Multi-head self-attention TRN2 kernel.

Sharding (8 cores): core c = (b, hg) with b = c // 4 (batch), hg = c % 4
(head group of 4 heads = 512 feature slice). Each core:
  - phase A: K^T, V projections for its 4 heads over its batch
  - phase B1: Q^T projection, spilled to DRAM
  - phase B2: flash-style attention per (head, s-tile): scores -> exp ->
    P@V with a ones-matmul denominator; normalization via PE-broadcast
    of the denominator + DVE reciprocal/multiply
  - per-head AllGather of O^T across the 4 cores of its batch group
  - phase C: out-projection for its 512-column output slice + bo
Host assembles the two batches x four column slices (pure concatenation).

Matmuls run in bf16 (fp32 PSUM accumulation; ~3.6e-3 rel err vs the fp32
reference, dominated by operand rounding). The softmax skips the
max-subtraction: scores*scale here are within [-2, 2], far from exp range
limits, and softmax is shift-invariant.
"""

import sys

sys.path.insert(0, "/opt/trn_rl_repo")

import ml_dtypes
import numpy as np

import concourse.bass as bass
import concourse.mybir as mybir
import concourse.tile as tile
from concourse.bass_utils import run_bass_kernel_spmd

F32 = mybir.dt.float32
F32R = mybir.dt.float32r
BF16 = mybir.dt.bfloat16
ID = mybir.ActivationFunctionType.Identity
EXP = mybir.ActivationFunctionType.Exp

P = 128          # partitions
D = 2048         # hidden
S = 2048         # sequence
B = 2            # batch
HPC = 4          # heads per core
E = 512          # feature slice per core (4 heads * 128)
ST = 512         # s-tile width
N_ST = S // ST           # 4 s-tiles
N_DC = D // P            # 16 contraction chunks
N_TC = S // P            # 16 t-chunks (keys)
N_SS = S // P            # 16 s-strips (phase C)
SCALE = 1.0 / np.sqrt(128.0)

_CACHE = {}


def _install_ntff_hook():
    """Recreate the missing antenv.axon_hooks module so trace=True works."""
    import types
    import ctypes
    import contextlib

    if "antenv.axon_hooks" in sys.modules:
        return
    lib = ctypes.CDLL("/opt/axon/libaxon_pjrt.so")
    if not hasattr(lib, "axon_start_nrt_profile"):
        return
    lib.axon_start_nrt_profile.argtypes = [
        ctypes.POINTER(ctypes.c_int64), ctypes.c_size_t]
    lib.axon_start_nrt_profile.restype = ctypes.c_int64
    lib.axon_stop_nrt_profile.argtypes = [ctypes.c_char_p]
    lib.axon_stop_nrt_profile.restype = ctypes.c_int64

    @contextlib.contextmanager
    def _hook(output_dir, device_ids):
        import jax
        jax.devices()
        if device_ids:
            ids = (ctypes.c_int64 * len(device_ids))(*device_ids)
            rc = lib.axon_start_nrt_profile(ids, len(device_ids))
        else:
            rc = lib.axon_start_nrt_profile(None, 0)
        if rc != 0:
            raise RuntimeError(f"axon_start_nrt_profile rc={rc}")
        try:
            yield
        finally:
            n = lib.axon_stop_nrt_profile(str(output_dir).encode())
            print(f"profile: {n} file(s) written to {output_dir}",
                  file=sys.stderr)

    mod = types.ModuleType("antenv.axon_hooks")
    _state = {"hook": _hook}
    mod.set_axon_ntff_profile_hook = lambda h: _state.__setitem__("hook", h)
    mod.get_axon_ntff_profile_hook = lambda: _state["hook"]
    sys.modules["antenv.axon_hooks"] = mod
    import antenv
    antenv.axon_hooks = mod


def split_multi_waits(nc, limit=1):
    """This container's walrus accepts only `limit` sync waits per
    instruction; hoist extras onto single-wait NoOps on the same engine."""
    for fn in nc.m.functions:
        for bb in fn.blocks:
            new_insts = []
            for inst in bb.instructions:
                si = inst.sync_info
                nw = len(si.on_wait) if si and si.on_wait else 0
                if nw > limit:
                    waits = list(si.on_wait)
                    head, tail = waits[:-limit], waits[-limit:]
                    for j, w in enumerate(head):
                        nop = mybir.InstNoOp(
                            name=f"{inst.name}-wsplit{j}", ins=[], outs=[])
                        nop.engine = inst.engine
                        nop.sync_info = mybir.SyncInfo(on_wait=[w], on_update=[])
                        new_insts.append(nop)
                    inst.sync_info = mybir.SyncInfo(
                        on_wait=tail, on_update=list(si.on_update or []))
                new_insts.append(inst)
            bb.instructions = new_insts


def build_nc():
    nc = bass.Bass()

    xt_ext = nc.declare_dram_parameter("xt", [D, S], BF16, isOutput=False)
    wq_ext = nc.declare_dram_parameter("wq", [D, E], BF16, isOutput=False)
    wk_ext = nc.declare_dram_parameter("wk", [D, E], BF16, isOutput=False)
    wv_ext = nc.declare_dram_parameter("wv", [D, E], BF16, isOutput=False)
    wo_ext = nc.declare_dram_parameter("wo", [D, E], BF16, isOutput=False)
    bq_ext = nc.declare_dram_parameter("bq", [P, HPC], F32, isOutput=False)
    bk_ext = nc.declare_dram_parameter("bk", [P, HPC], F32, isOutput=False)
    bv_ext = nc.declare_dram_parameter("bv", [P, E], F32, isOutput=False)
    bo_ext = nc.declare_dram_parameter("bo", [P, E], F32, isOutput=False)
    onesc_ext = nc.declare_dram_parameter("onesc", [P, 1], BF16, isOutput=False)
    onesr_ext = nc.declare_dram_parameter("onesr", [1, P], F32R, isOutput=False)
    out_ext = nc.declare_dram_parameter("out", [S, E], F32, isOutput=True)

    xt_r = xt_ext.rearrange("(dc p) s -> p dc s", p=P)
    w_r = {
        "wq": wq_ext.rearrange("(dc p) e -> p dc e", p=P),
        "wk": wk_ext.rearrange("(dc p) e -> p dc e", p=P),
        "wv": wv_ext.rearrange("(dc p) e -> p dc e", p=P),
        "wo": wo_ext.rearrange("(dc p) e -> p dc e", p=P),
    }

    with tile.TileContext(nc) as tc:
        with tc.tile_pool(name="persist", bufs=1) as persist, \
             tc.tile_pool(name="xp", bufs=4) as xp, \
             tc.tile_pool(name="dram", bufs=1, space="DRAM") as dram:

            # ---- constants / biases ----
            bq_sb = persist.tile([P, HPC], F32)
            bk_sb = persist.tile([P, HPC], F32)
            bv_sb = persist.tile([P, E], F32)
            bo_sb = persist.tile([P, E], F32)
            onesc = persist.tile([P, 1], BF16)
            onesr = persist.tile([1, P], F32R)
            nc.sync.dma_start(bq_sb[:], bq_ext[:])
            nc.sync.dma_start(bk_sb[:], bk_ext[:])
            nc.sync.dma_start(bv_sb[:], bv_ext[:])
            nc.sync.dma_start(bo_sb[:], bo_ext[:])
            nc.sync.dma_start(onesc[:], onesc_ext[:])
            nc.sync.dma_start(onesr[:], onesr_ext[:])

            # ---- persistent activations ----
            k_sb = persist.tile([P, HPC, S], BF16)   # K^T [dh, h, t]
            v_sb = persist.tile([P, N_TC, E], BF16)  # V   [t-strip, tc, e]

            ag_in = [dram.tile([P, S], BF16, name=f"ag_in{h}")
                     for h in range(HPC)]
            ag_out = [dram.tile([4 * P, S], BF16, name=f"ag_out{h}")
                      for h in range(HPC)]
            q_dram = dram.tile([E, S], BF16, name="q_dram")

            # ================= Phase A: K^T and V projections =================
            with tc.tile_pool(name="wkv", bufs=1) as wkv, \
                 tc.tile_pool(name="psA", bufs=4, space="PSUM") as psA:
                wk_sb = wkv.tile([P, N_DC, E], BF16)
                wv_sb = wkv.tile([P, N_DC, E], BF16)
                nc.sync.dma_start(wk_sb[:], w_r["wk"])
                nc.sync.dma_start(wv_sb[:], w_r["wv"])

                for st in range(N_ST):
                    xt_sb = xp.tile([P, N_DC, ST], BF16, tag="xt")
                    nc.sync.dma_start(xt_sb[:], xt_r[:, :, st * ST:(st + 1) * ST])
                    # K^T strips: out [e-strip 128, s 512]
                    for es in range(HPC):
                        psk = psA.tile([P, ST], F32, tag="psA")
                        for dc in range(N_DC):
                            nc.tensor.matmul(
                                psk[:], wk_sb[:, dc, es * P:(es + 1) * P],
                                xt_sb[:, dc],
                                start=(dc == 0), stop=(dc == N_DC - 1))
                        with nc.allow_low_precision(reason="bf16 K"):
                            nc.scalar.activation(
                                k_sb[:, es, st * ST:(st + 1) * ST], psk[:],
                                ID, bias=bk_sb[:, es:es + 1], scale=1.0)
                    # V strips: out [t-strip 128, e 512]
                    for tl in range(ST // P):
                        ts_g = st * (ST // P) + tl
                        psv = psA.tile([P, E], F32, tag="psA")
                        for dc in range(N_DC):
                            nc.tensor.matmul(
                                psv[:], xt_sb[:, dc, tl * P:(tl + 1) * P],
                                wv_sb[:, dc],
                                start=(dc == 0), stop=(dc == N_DC - 1))
                        with nc.allow_low_precision(reason="bf16 V"):
                            nc.vector.tensor_add(
                                out=v_sb[:, ts_g, :], in0=psv[:], in1=bv_sb[:])

            # ============ Phase B1: Q^T projection, spilled to DRAM ============
            with tc.tile_pool(name="wq", bufs=1) as wqp, \
                 tc.tile_pool(name="workB1", bufs=3) as workB1, \
                 tc.tile_pool(name="psq", bufs=3, space="PSUM") as psq_pool:
                wq_sb = wqp.tile([P, N_DC, E], BF16)
                nc.sync.dma_start(wq_sb[:], w_r["wq"])

                for st in range(N_ST):
                    xt_sb = xp.tile([P, N_DC, ST], BF16, tag="xt")
                    nc.sync.dma_start(xt_sb[:], xt_r[:, :, st * ST:(st + 1) * ST])
                    for es in range(HPC):
                        psq = psq_pool.tile([P, ST], F32, tag="psq")
                        for dc in range(N_DC):
                            nc.tensor.matmul(
                                psq[:], wq_sb[:, dc, es * P:(es + 1) * P],
                                xt_sb[:, dc],
                                start=(dc == 0), stop=(dc == N_DC - 1))
                        q_stage = workB1.tile([P, ST], BF16, tag="qs")
                        with nc.allow_low_precision(reason="bf16 Q"):
                            nc.scalar.activation(
                                q_stage[:], psq[:],
                                ID, bias=bq_sb[:, es:es + 1], scale=1.0)
                        nc.sync.dma_start(
                            q_dram[es * P:(es + 1) * P, st * ST:(st + 1) * ST],
                            q_stage[:])

            # ===== Phase B2: attention, head-outer =====
            with tc.tile_pool(name="workB2", bufs=3) as work, \
                 tc.tile_pool(name="pssc", bufs=2, space="PSUM") as pssc, \
                 tc.tile_pool(name="psB", bufs=1, space="PSUM") as psB:
                for h in range(HPC):
                    for st in range(N_ST):
                        q_t = work.tile([P, ST], BF16, tag="qb")
                        nc.sync.dma_start(
                            q_t[:],
                            q_dram[h * P:(h + 1) * P, st * ST:(st + 1) * ST])
                        pso = psB.tile([P, ST], F32, tag="o", bufs=2)
                        psd = psB.tile([1, ST], F32, tag="d", bufs=1)
                        for tc2 in range(N_TC // 2):
                            t0, t1 = 2 * tc2, 2 * tc2 + 1
                            pss = pssc.tile([P, 2 * ST], F32, tag="sc")
                            nc.tensor.matmul(
                                pss[:, :ST], k_sb[:, h, t0 * P:(t0 + 1) * P],
                                q_t[:], start=True, stop=True)
                            nc.tensor.matmul(
                                pss[:, ST:], k_sb[:, h, t1 * P:(t1 + 1) * P],
                                q_t[:], start=True, stop=True)
                            pt = work.tile([P, 2 * ST], BF16, tag="pt")
                            with nc.allow_low_precision(reason="bf16 P"):
                                nc.scalar.activation(pt[:], pss[:], EXP,
                                                     bias=0.0, scale=float(SCALE))
                            nc.tensor.matmul(
                                pso[:], v_sb[:, t0, h * P:(h + 1) * P],
                                pt[:, :ST],
                                start=(tc2 == 0), stop=False)
                            nc.tensor.matmul(
                                pso[:], v_sb[:, t1, h * P:(h + 1) * P],
                                pt[:, ST:],
                                start=False, stop=(tc2 == N_TC // 2 - 1))
                            nc.tensor.matmul(
                                psd[:], onesc[:], pt[:, :ST],
                                start=(tc2 == 0), stop=False)
                            nc.tensor.matmul(
                                psd[:], onesc[:], pt[:, ST:],
                                start=False, stop=(tc2 == N_TC // 2 - 1))
                        # normalize: PE work first (broadcast), DVE tail
                        # overlaps the next block's matmuls
                        dsb = work.tile([1, ST], F32R, tag="dsb")
                        with nc.allow_low_precision(reason="denom stage"):
                            nc.vector.tensor_copy(dsb[:], psd[:])
                        rb_ps = psB.tile([P, ST], F32, tag="rb", bufs=1)
                        nc.tensor.matmul(rb_ps[:], onesr[:], dsb[:],
                                         start=True, stop=True)
                        rb_sb = work.tile([P, ST], F32, tag="rb_sb")
                        nc.vector.tensor_copy(rb_sb[:], rb_ps[:])
                        rcp = work.tile([P, ST], F32, tag="rcp")
                        nc.vector.reciprocal(rcp[:], rb_sb[:])
                        o_sb = work.tile([P, ST], BF16, tag="o_sb")
                        with nc.allow_low_precision(reason="bf16 O"):
                            nc.vector.tensor_mul(out=o_sb[:], in0=pso[:],
                                                 in1=rcp[:])
                        nc.sync.dma_start(
                            ag_in[h][:, st * ST:(st + 1) * ST], o_sb[:])
                    # AllGather head h across the batch group
                    nc.gpsimd.collective_compute(
                        "AllGather", mybir.AluOpType.bypass,
                        ins=[ag_in[h][:]], outs=[ag_out[h][:]],
                        replica_groups=[[0, 1, 2, 3], [4, 5, 6, 7]],
                    )

            # ================= Phase C: out projection =================
            with tc.tile_pool(name="wo", bufs=1) as wop, \
                 tc.tile_pool(name="workC", bufs=3) as work, \
                 tc.tile_pool(name="psC", bufs=4, space="PSUM") as psC:
                wo_sb = wop.tile([P, N_DC, E], BF16)
                nc.sync.dma_start(wo_sb[:], w_r["wo"])
                ag_r = [ag_out[h].rearrange("(g p) s -> p g s", p=P)
                        for h in range(HPC)]
                for ss in range(N_SS):
                    of_sb = work.tile([P, HPC, 4, P], BF16, tag="of")
                    for h in range(HPC):
                        nc.sync.dma_start(
                            of_sb[:, h, :, :],
                            ag_r[h][:, :, ss * P:(ss + 1) * P])
                    psc = psC.tile([P, E], F32, tag="psC")
                    for ec in range(N_DC):
                        hg, hh = ec // 4, ec % 4
                        nc.tensor.matmul(
                            psc[:], of_sb[:, hh, hg, :], wo_sb[:, ec, :],
                            start=(ec == 0), stop=(ec == N_DC - 1))
                    out_sb = work.tile([P, E], F32, tag="out_sb")
                    nc.vector.tensor_add(out=out_sb[:], in0=psc[:], in1=bo_sb[:])
                    nc.sync.dma_start(out_ext[ss * P:(ss + 1) * P, :], out_sb[:])

    split_multi_waits(nc)
    return nc


def _get_nc():
    if "nc" not in _CACHE:
        _CACHE["nc"] = build_nc()
    return _CACHE["nc"]


def _prep_in_maps(X, Wq, bq, Wk, bk, Wv, bv, Wo, bo):
    bf16 = ml_dtypes.bfloat16
    xt = [np.ascontiguousarray(X[b].T).astype(bf16) for b in range(B)]
    onesc = np.ones((P, 1), bf16)
    onesr = np.ones((1, P), np.float32)
    in_maps = []
    for c in range(8):
        b, hg = c // 4, c % 4
        sl = slice(hg * E, (hg + 1) * E)
        in_maps.append({
            "xt": xt[b],
            "wq": np.ascontiguousarray(Wq[sl, :].T).astype(bf16),
            "wk": np.ascontiguousarray(Wk[sl, :].T).astype(bf16),
            "wv": np.ascontiguousarray(Wv[sl, :].T).astype(bf16),
            "wo": np.ascontiguousarray(Wo[sl, :].T).astype(bf16),
            "bq": np.ascontiguousarray(bq[sl].reshape(HPC, P).T),
            "bk": np.ascontiguousarray(bk[sl].reshape(HPC, P).T),
            "bv": np.broadcast_to(bv[sl], (P, E)).copy(),
            "bo": np.broadcast_to(bo[sl], (P, E)).copy(),
            "onesc": onesc,
            "onesr": onesr,
        })
    return in_maps


def kernel(X, Wq, bq, Wk, bk, Wv, bv, Wo, bo, _trace=False):
    X = np.asarray(X, dtype=np.float32)
    Wq = np.asarray(Wq, dtype=np.float32)
    bq = np.asarray(bq, dtype=np.float32)
    Wk = np.asarray(Wk, dtype=np.float32)
    bk = np.asarray(bk, dtype=np.float32)
    Wv = np.asarray(Wv, dtype=np.float32)
    bv = np.asarray(bv, dtype=np.float32)
    Wo = np.asarray(Wo, dtype=np.float32)
    bo = np.asarray(bo, dtype=np.float32)

    nc = _get_nc()
    in_maps = _prep_in_maps(X, Wq, bq, Wk, bk, Wv, bv, Wo, bo)
    if _trace:
        _install_ntff_hook()
    res = run_bass_kernel_spmd(nc, in_maps, core_ids=list(range(8)),
                               trace=_trace)
    if _trace:
        _CACHE["last_results"] = res

    out = np.empty((B, S, D), dtype=np.float32)
    for c in range(8):
        b, hg = c // 4, c % 4
        out[b, :, hg * E:(hg + 1) * E] = res.results[c]["out"]
    return out


# revision 11
# speedup vs baseline: 1.5122x; 1.1306x over previous
"""Multi-head self-attention TRN2 kernel.

Sharding (8 cores): core c = (b, hg) with b = c // 4 (batch), hg = c % 4
(head group of 4 heads = 512 feature slice). Each core:
  - phase A: K^T, V projections for its 4 heads over its batch
  - phase B1: Q^T projection, spilled to DRAM
  - phase B2: flash-style attention per (head, s-tile): scores -> exp ->
    P@V with a ones-matmul denominator; normalization via PE-broadcast
    of the denominator + DVE reciprocal/multiply
  - per-head AllGather of O^T across the 4 cores of its batch group
  - phase C: out-projection for its 512-column output slice + bo
Host assembles the two batches x four column slices (pure concatenation).

Matmuls run in bf16 (fp32 PSUM accumulation; ~3.6e-3 rel err vs the fp32
reference, dominated by operand rounding). The softmax skips the
max-subtraction: scores*scale here are within [-2, 2], far from exp range
limits, and softmax is shift-invariant.
"""

import sys

sys.path.insert(0, "/opt/trn_rl_repo")

import ml_dtypes
import numpy as np

import concourse.bass as bass
import concourse.mybir as mybir
import concourse.tile as tile
from concourse.bass_utils import run_bass_kernel_spmd

F32 = mybir.dt.float32
F32R = mybir.dt.float32r
BF16 = mybir.dt.bfloat16
ID = mybir.ActivationFunctionType.Identity
EXP = mybir.ActivationFunctionType.Exp

P = 128          # partitions
D = 2048         # hidden
S = 2048         # sequence
B = 2            # batch
HPC = 4          # heads per core
E = 512          # feature slice per core (4 heads * 128)
ST = 512         # s-tile width
N_ST = S // ST           # 4 s-tiles
N_DC = D // P            # 16 contraction chunks
N_TC = S // P            # 16 t-chunks (keys)
N_SS = S // P            # 16 s-strips (phase C)
SCALE = 1.0 / np.sqrt(128.0)

_CACHE = {}


def _install_ntff_hook():
    """Recreate the missing antenv.axon_hooks module so trace=True works."""
    import types
    import ctypes
    import contextlib

    if "antenv.axon_hooks" in sys.modules:
        return
    lib = ctypes.CDLL("/opt/axon/libaxon_pjrt.so")
    if not hasattr(lib, "axon_start_nrt_profile"):
        return
    lib.axon_start_nrt_profile.argtypes = [
        ctypes.POINTER(ctypes.c_int64), ctypes.c_size_t]
    lib.axon_start_nrt_profile.restype = ctypes.c_int64
    lib.axon_stop_nrt_profile.argtypes = [ctypes.c_char_p]
    lib.axon_stop_nrt_profile.restype = ctypes.c_int64

    @contextlib.contextmanager
    def _hook(output_dir, device_ids):
        import jax
        jax.devices()
        if device_ids:
            ids = (ctypes.c_int64 * len(device_ids))(*device_ids)
            rc = lib.axon_start_nrt_profile(ids, len(device_ids))
        else:
            rc = lib.axon_start_nrt_profile(None, 0)
        if rc != 0:
            raise RuntimeError(f"axon_start_nrt_profile rc={rc}")
        try:
            yield
        finally:
            n = lib.axon_stop_nrt_profile(str(output_dir).encode())
            print(f"profile: {n} file(s) written to {output_dir}",
                  file=sys.stderr)

    mod = types.ModuleType("antenv.axon_hooks")
    _state = {"hook": _hook}
    mod.set_axon_ntff_profile_hook = lambda h: _state.__setitem__("hook", h)
    mod.get_axon_ntff_profile_hook = lambda: _state["hook"]
    sys.modules["antenv.axon_hooks"] = mod
    import antenv
    antenv.axon_hooks = mod


def split_multi_waits(nc, limit=1):
    """This container's walrus accepts only `limit` sync waits per
    instruction; hoist extras onto single-wait NoOps on the same engine."""
    for fn in nc.m.functions:
        for bb in fn.blocks:
            new_insts = []
            for inst in bb.instructions:
                si = inst.sync_info
                nw = len(si.on_wait) if si and si.on_wait else 0
                if nw > limit:
                    waits = list(si.on_wait)
                    head, tail = waits[:-limit], waits[-limit:]
                    for j, w in enumerate(head):
                        nop = mybir.InstNoOp(
                            name=f"{inst.name}-wsplit{j}", ins=[], outs=[])
                        nop.engine = inst.engine
                        nop.sync_info = mybir.SyncInfo(on_wait=[w], on_update=[])
                        new_insts.append(nop)
                    inst.sync_info = mybir.SyncInfo(
                        on_wait=tail, on_update=list(si.on_update or []))
                new_insts.append(inst)
            bb.instructions = new_insts


def build_nc():
    nc = bass.Bass()

    xt_ext = nc.declare_dram_parameter("xt", [D, S], BF16, isOutput=False)
    wq_ext = nc.declare_dram_parameter("wq", [D, E], BF16, isOutput=False)
    wk_ext = nc.declare_dram_parameter("wk", [D, E], BF16, isOutput=False)
    wv_ext = nc.declare_dram_parameter("wv", [D, E], BF16, isOutput=False)
    wo_ext = nc.declare_dram_parameter("wo", [D, E], BF16, isOutput=False)
    bq_ext = nc.declare_dram_parameter("bq", [P, HPC], F32, isOutput=False)
    bk_ext = nc.declare_dram_parameter("bk", [P, HPC], F32, isOutput=False)
    bv_ext = nc.declare_dram_parameter("bv", [P, E], F32, isOutput=False)
    bo_ext = nc.declare_dram_parameter("bo", [P, E], F32, isOutput=False)
    ident_ext = nc.declare_dram_parameter("ident", [P, P], BF16, isOutput=False)
    out_ext = nc.declare_dram_parameter("out", [S, E], F32, isOutput=True)

    xt_r = xt_ext.rearrange("(dc p) s -> p dc s", p=P)
    w_r = {
        "wq": wq_ext.rearrange("(dc p) e -> p dc e", p=P),
        "wk": wk_ext.rearrange("(dc p) e -> p dc e", p=P),
        "wv": wv_ext.rearrange("(dc p) e -> p dc e", p=P),
        "wo": wo_ext.rearrange("(dc p) e -> p dc e", p=P),
    }

    with tile.TileContext(nc) as tc:
        with tc.tile_pool(name="persist", bufs=1) as persist, \
             tc.tile_pool(name="xp", bufs=4) as xp, \
             tc.tile_pool(name="dram", bufs=1, space="DRAM") as dram:

            # ---- constants / biases ----
            bq_sb = persist.tile([P, HPC], F32)
            bk_sb = persist.tile([P, HPC], F32)
            bv_sb = persist.tile([P, E], F32)
            bo_sb = persist.tile([P, E], F32)
            ident = persist.tile([P, P], BF16)
            nc.sync.dma_start(bq_sb[:], bq_ext[:])
            nc.sync.dma_start(bk_sb[:], bk_ext[:])
            nc.sync.dma_start(bv_sb[:], bv_ext[:])
            nc.sync.dma_start(bo_sb[:], bo_ext[:])
            nc.sync.dma_start(ident[:], ident_ext[:])

            # ---- persistent activations ----
            k_sb = persist.tile([P, HPC, S], BF16)     # K^T [dh, h, t]
            # V plus a trailing ones column per head: [t-strip, tc, h, dh+1]
            v_sb = persist.tile([P, N_TC, HPC * (P + 1)], BF16)
            v_4d = v_sb.rearrange("p tc (h w) -> p tc h w", w=P + 1)
            nc.vector.memset(v_4d[:, :, :, P:P + 1], 1.0)

            ag_in = [dram.tile([P, S], BF16, name=f"ag_in{h}")
                     for h in range(HPC)]
            ag_out = [dram.tile([4 * P, S], BF16, name=f"ag_out{h}")
                      for h in range(HPC)]
            q_dram = dram.tile([E, S], BF16, name="q_dram")

            # ================= Phase A: K^T and V projections =================
            with tc.tile_pool(name="wkv", bufs=1) as wkv, \
                 tc.tile_pool(name="psA", bufs=4, space="PSUM") as psA:
                wk_sb = wkv.tile([P, N_DC, E], BF16)
                wv_sb = wkv.tile([P, N_DC, E], BF16)
                nc.sync.dma_start(wk_sb[:], w_r["wk"])
                nc.sync.dma_start(wv_sb[:], w_r["wv"])

                for st in range(N_ST):
                    xt_sb = xp.tile([P, N_DC, ST], BF16, tag="xt")
                    nc.sync.dma_start(xt_sb[:], xt_r[:, :, st * ST:(st + 1) * ST])
                    # K^T strips: out [e-strip 128, s 512]
                    for es in range(HPC):
                        psk = psA.tile([P, ST], F32, tag="psA")
                        for dc in range(N_DC):
                            nc.tensor.matmul(
                                psk[:], wk_sb[:, dc, es * P:(es + 1) * P],
                                xt_sb[:, dc],
                                start=(dc == 0), stop=(dc == N_DC - 1))
                        with nc.allow_low_precision(reason="bf16 K"):
                            nc.scalar.activation(
                                k_sb[:, es, st * ST:(st + 1) * ST], psk[:],
                                ID, bias=bk_sb[:, es:es + 1], scale=1.0)
                    # V strips: out [t-strip 128, e 512]
                    for tl in range(ST // P):
                        ts_g = st * (ST // P) + tl
                        psv = psA.tile([P, E], F32, tag="psA")
                        for dc in range(N_DC):
                            nc.tensor.matmul(
                                psv[:], xt_sb[:, dc, tl * P:(tl + 1) * P],
                                wv_sb[:, dc],
                                start=(dc == 0), stop=(dc == N_DC - 1))
                        with nc.allow_low_precision(reason="bf16 V"):
                            nc.vector.tensor_add(
                                out=v_4d[:, ts_g, :, :P],
                                in0=psv.rearrange("p (h w) -> p h w", w=P),
                                in1=bv_sb.rearrange("p (h w) -> p h w", w=P))

            # ============ Phase B1: Q^T projection, spilled to DRAM ============
            with tc.tile_pool(name="wq", bufs=1) as wqp, \
                 tc.tile_pool(name="workB1", bufs=3) as workB1, \
                 tc.tile_pool(name="psq", bufs=3, space="PSUM") as psq_pool:
                wq_sb = wqp.tile([P, N_DC, E], BF16)
                nc.sync.dma_start(wq_sb[:], w_r["wq"])

                for st in range(N_ST):
                    xt_sb = xp.tile([P, N_DC, ST], BF16, tag="xt")
                    nc.sync.dma_start(xt_sb[:], xt_r[:, :, st * ST:(st + 1) * ST])
                    for es in range(HPC):
                        psq = psq_pool.tile([P, ST], F32, tag="psq")
                        for dc in range(N_DC):
                            nc.tensor.matmul(
                                psq[:], wq_sb[:, dc, es * P:(es + 1) * P],
                                xt_sb[:, dc],
                                start=(dc == 0), stop=(dc == N_DC - 1))
                        q_stage = workB1.tile([P, ST], BF16, tag="qs")
                        with nc.allow_low_precision(reason="bf16 Q"):
                            nc.scalar.activation(
                                q_stage[:], psq[:],
                                ID, bias=bq_sb[:, es:es + 1], scale=1.0)
                        nc.sync.dma_start(
                            q_dram[es * P:(es + 1) * P, st * ST:(st + 1) * ST],
                            q_stage[:])

            # ===== Phase B2: attention, head-outer =====
            # Per (head, s-tile) block: scores -> exp -> P@[V|1] in [s, dh+1]
            # orientation (last column accumulates the softmax denominator),
            # per-partition normalize, PE-transpose back to [dh, s] for the
            # AllGather layout.
            with tc.tile_pool(name="workB2", bufs=3) as work, \
                 tc.tile_pool(name="pssc", bufs=2, space="PSUM") as pssc, \
                 tc.tile_pool(name="psB", bufs=4, space="PSUM") as psB:
                for h in range(HPC):
                    for st in range(N_ST):
                        q_t = work.tile([P, ST], BF16, tag="qb")
                        nc.sync.dma_start(
                            q_t[:],
                            q_dram[h * P:(h + 1) * P, st * ST:(st + 1) * ST])
                        pts = []
                        for tc2 in range(N_TC // 2):
                            t0, t1 = 2 * tc2, 2 * tc2 + 1
                            pss = pssc.tile([P, 2 * ST], F32, tag="sc")
                            nc.tensor.matmul(
                                pss[:, :ST], k_sb[:, h, t0 * P:(t0 + 1) * P],
                                q_t[:], start=True, stop=True)
                            nc.tensor.matmul(
                                pss[:, ST:], k_sb[:, h, t1 * P:(t1 + 1) * P],
                                q_t[:], start=True, stop=True)
                            pt = work.tile([P, 2 * ST], BF16, tag="pt", bufs=10)
                            with nc.allow_low_precision(reason="bf16 P"):
                                nc.scalar.activation(pt[:], pss[:], EXP,
                                                     bias=0.0, scale=float(SCALE))
                            pts.append(pt)
                        agst = work.tile([P, ST], BF16, tag="agst")
                        for j in range(ST // P):
                            po = psB.tile([P, P + 1], F32, tag="ot")
                            for tc2 in range(N_TC // 2):
                                for half in range(2):
                                    tcI = 2 * tc2 + half
                                    lhsT = pts[tc2][:, half * ST + j * P:
                                                    half * ST + (j + 1) * P]
                                    nc.tensor.matmul(
                                        po[:], lhsT,
                                        v_sb[:, tcI,
                                             h * (P + 1):(h + 1) * (P + 1)],
                                        start=(tcI == 0),
                                        stop=(tcI == N_TC - 1))
                            rcp = work.tile([P, 1], F32, tag="rcp")
                            nc.vector.reciprocal(rcp[:], po[:, P:P + 1])
                            o_str = work.tile([P, P], BF16, tag="ostr")
                            with nc.allow_low_precision(reason="bf16 O"):
                                nc.vector.tensor_scalar_mul(
                                    o_str[:], po[:, :P], rcp[:, 0:1])
                            pot = psB.tile([P, P], BF16, tag="ot")
                            nc.tensor.transpose(pot[:], o_str[:], ident[:])
                            nc.vector.tensor_copy(
                                agst[:, j * P:(j + 1) * P], pot[:])
                        nc.sync.dma_start(
                            ag_in[h][:, st * ST:(st + 1) * ST], agst[:])
                    # AllGather head h across the batch group
                    nc.gpsimd.collective_compute(
                        "AllGather", mybir.AluOpType.bypass,
                        ins=[ag_in[h][:]], outs=[ag_out[h][:]],
                        replica_groups=[[0, 1, 2, 3], [4, 5, 6, 7]],
                    )

            # ================= Phase C: out projection =================
            with tc.tile_pool(name="wo", bufs=1) as wop, \
                 tc.tile_pool(name="workC", bufs=3) as work, \
                 tc.tile_pool(name="psC", bufs=4, space="PSUM") as psC:
                wo_sb = wop.tile([P, N_DC, E], BF16)
                nc.sync.dma_start(wo_sb[:], w_r["wo"])
                ag_r = [ag_out[h].rearrange("(g p) s -> p g s", p=P)
                        for h in range(HPC)]
                for ss in range(N_SS):
                    of_sb = work.tile([P, HPC, 4, P], BF16, tag="of")
                    for h in range(HPC):
                        nc.sync.dma_start(
                            of_sb[:, h, :, :],
                            ag_r[h][:, :, ss * P:(ss + 1) * P])
                    psc = psC.tile([P, E], F32, tag="psC")
                    for ec in range(N_DC):
                        hg, hh = ec // 4, ec % 4
                        nc.tensor.matmul(
                            psc[:], of_sb[:, hh, hg, :], wo_sb[:, ec, :],
                            start=(ec == 0), stop=(ec == N_DC - 1))
                    out_sb = work.tile([P, E], F32, tag="out_sb")
                    nc.vector.tensor_add(out=out_sb[:], in0=psc[:], in1=bo_sb[:])
                    nc.sync.dma_start(out_ext[ss * P:(ss + 1) * P, :], out_sb[:])

    split_multi_waits(nc)
    return nc


def _get_nc():
    if "nc" not in _CACHE:
        _CACHE["nc"] = build_nc()
    return _CACHE["nc"]


def _prep_in_maps(X, Wq, bq, Wk, bk, Wv, bv, Wo, bo):
    bf16 = ml_dtypes.bfloat16
    xt = [np.ascontiguousarray(X[b].T).astype(bf16) for b in range(B)]
    ident = np.eye(P, dtype=bf16)
    in_maps = []
    for c in range(8):
        b, hg = c // 4, c % 4
        sl = slice(hg * E, (hg + 1) * E)
        in_maps.append({
            "xt": xt[b],
            "wq": np.ascontiguousarray(Wq[sl, :].T).astype(bf16),
            "wk": np.ascontiguousarray(Wk[sl, :].T).astype(bf16),
            "wv": np.ascontiguousarray(Wv[sl, :].T).astype(bf16),
            "wo": np.ascontiguousarray(Wo[sl, :].T).astype(bf16),
            "bq": np.ascontiguousarray(bq[sl].reshape(HPC, P).T),
            "bk": np.ascontiguousarray(bk[sl].reshape(HPC, P).T),
            "bv": np.broadcast_to(bv[sl], (P, E)).copy(),
            "bo": np.broadcast_to(bo[sl], (P, E)).copy(),
            "ident": ident,
        })
    return in_maps


def kernel(X, Wq, bq, Wk, bk, Wv, bv, Wo, bo, _trace=False):
    X = np.asarray(X, dtype=np.float32)
    Wq = np.asarray(Wq, dtype=np.float32)
    bq = np.asarray(bq, dtype=np.float32)
    Wk = np.asarray(Wk, dtype=np.float32)
    bk = np.asarray(bk, dtype=np.float32)
    Wv = np.asarray(Wv, dtype=np.float32)
    bv = np.asarray(bv, dtype=np.float32)
    Wo = np.asarray(Wo, dtype=np.float32)
    bo = np.asarray(bo, dtype=np.float32)

    nc = _get_nc()
    in_maps = _prep_in_maps(X, Wq, bq, Wk, bk, Wv, bv, Wo, bo)
    if _trace:
        _install_ntff_hook()
    res = run_bass_kernel_spmd(nc, in_maps, core_ids=list(range(8)),
                               trace=_trace)
    if _trace:
        _CACHE["last_results"] = res

    out = np.empty((B, S, D), dtype=np.float32)
    for c in range(8):
        b, hg = c // 4, c % 4
        out[b, :, hg * E:(hg + 1) * E] = res.results[c]["out"]
    return out
